# revision 9
# baseline (speedup 1.0000x reference)
"""Trainium2 Bass kernel for BaseLayerWithLoRA: out = x @ W.T + b + (x @ A.T) @ B.T.

Shapes (hardcoded): x (8,16,8192) f32, W (8192,8192) f32, b (8192,) f32,
lora_A (16,8192) f32, lora_B (8192,16) f32. Output (8,16,8192) f32.

Strategy: tensor-parallel over out_features (Dout=8192) across 8 cores,
1024 outputs per core; x / lora_A replicated. All matmul operands cast to
fp16 on host (PSUM accumulates fp32; measured rel err ~3e-4). Host
pre-transposes x, lora_A, W so every DMA is a contiguous partition-major
load; bias is folded into the LoRA matmul as a rank-1 term with a
constant-ones row.
"""

import sys

for p in ("/opt/trn_rl_repo",):
    if p not in sys.path:
        sys.path.insert(0, p)

import numpy as np

import concourse.bacc as bacc
import concourse.bass as bass
import concourse.mybir as mybir
import concourse.tile as tile
from concourse.bass_utils import run_bass_kernel_spmd

# Problem constants
T = 128          # tokens = 8*16
DIN = 8192
DOUT = 8192
R = 16           # lora rank
NCORES = 8
DC = DOUT // NCORES      # 1024 out-features per core
KT = DIN // 128          # 64 k-tiles
KCHUNK = 4               # k-tiles per W DMA (1 MiB per transfer)
NCHUNK = KT // KCHUNK    # 16 W DMAs
F16 = mybir.dt.float16
F32 = mybir.dt.float32

_CACHE = {}
LAST_RESULT = None


def build_bass():
    nc = bacc.Bacc("TRN2", target_bir_lowering=False)
    xt_d = nc.dram_tensor("xt", [128, KT, T], F16, kind="ExternalInput")
    at_d = nc.dram_tensor("at", [128, KT, R], F16, kind="ExternalInput")
    wt_d = nc.dram_tensor("wt", [NCHUNK, 128, KCHUNK * DC], F16, kind="ExternalInput")
    bb_d = nc.dram_tensor("bb", [R + 1, DC], F16, kind="ExternalInput")
    out_d = nc.dram_tensor("out", [T, DC], F32, kind="ExternalOutput")

    with tile.TileContext(nc) as tc:
        with (
            tc.tile_pool(name="res", bufs=1) as res,
            tc.tile_pool(name="wts", bufs=8) as wts,
            tc.tile_pool(name="outs", bufs=2) as outs,
            tc.tile_pool(name="ps", bufs=1, space="PSUM") as ps,
        ):
            # Resident tiles: transposed x, transposed lora_A, [lora_B.T; b].
            # W streams on the SP HWDGE ring (nc.sync) right behind the small
            # critical tiles (at, bb, first quarter of xt); the rest of xt
            # rides the ACT ring (nc.scalar) in parallel.
            at_s = res.tile([128, KT, R], F16)
            nc.sync.dma_start(out=at_s[:], in_=at_d[:, :, :])
            bb_s = res.tile([R + 1, DC], F16)
            nc.sync.dma_start(out=bb_s[:], in_=bb_d[:, :])
            xt_s = res.tile([128, KT, T], F16)
            nc.sync.dma_start(out=xt_s[:, 0 : KT // 4, :], in_=xt_d[:, 0 : KT // 4, :])
            for c in range(1, 4):
                sl = slice(c * (KT // 4), (c + 1) * (KT // 4))
                nc.scalar.dma_start(out=xt_s[:, sl, :], in_=xt_d[:, sl, :])

            psum0 = ps.tile([T, 512], F32, tag="p0")
            psum1 = ps.tile([T, 512], F32, tag="p1")
            psum_xa = ps.tile([R, T], F32, tag="pxa")

            # Front-load the 64 xa matmuls (5 per W chunk) so the lora path
            # finishes mid-kernel instead of serializing into the tail.
            xa_sched = [
                list(range(c * 5, min(c * 5 + 5, KT))) for c in range(NCHUNK)
            ]
            for c in range(NCHUNK):
                wt_t = wts.tile([128, KCHUNK * DC], F16, tag="wt")
                nc.sync.dma_start(out=wt_t[:], in_=wt_d[c])
                for s in range(KCHUNK):
                    k = c * KCHUNK + s
                    lhs = xt_s[:, k, :]
                    nc.tensor.matmul(
                        psum0[:], lhs, wt_t[:, s * DC : s * DC + 512],
                        start=(k == 0), stop=False, skip_group_check=True,
                    )
                    nc.tensor.matmul(
                        psum1[:], lhs, wt_t[:, s * DC + 512 : (s + 1) * DC],
                        start=(k == 0), stop=False, skip_group_check=True,
                    )
                for kx in xa_sched[c]:
                    nc.tensor.matmul(
                        psum_xa[:], at_s[:, kx, :], xt_s[:, kx, :],
                        start=(kx == 0), stop=(kx == KT - 1), skip_group_check=True,
                    )

            # xa_aug rows 0..15 = (x @ A.T).T cast to fp16, row 16 = ones
            # (folds the bias add into the final matmul).
            xa_aug = res.tile([R + 1, T], F16)
            nc.vector.memset(xa_aug[:, :], 1.0)
            nc.any.tensor_copy(xa_aug[0:R, :], psum_xa[:])

            for n, psum in enumerate((psum0, psum1)):
                nc.tensor.matmul(
                    psum[:], xa_aug[:], bb_s[:, n * 512 : (n + 1) * 512],
                    start=False, stop=True, skip_group_check=True,
                )
                ot = outs.tile([T, 512], F32, tag="ot")
                nc.any.tensor_copy(ot[:], psum[:])
                nc.scalar.dma_start(out=out_d[:, n * 512 : (n + 1) * 512], in_=ot[:])

    nc.compile()
    return nc


def _prep_inputs(x, W, b, lora_A, lora_B):
    xf = np.asarray(x, dtype=np.float32).reshape(T, DIN)
    # xt[p, k, t] = x[t, 128k+p]
    xt = np.ascontiguousarray(
        xf.reshape(T, KT, 128).transpose(2, 1, 0).astype(np.float16)
    )
    # at[p, k, r] = A[r, 128k+p]
    at = np.ascontiguousarray(
        np.asarray(lora_A, np.float32).reshape(R, KT, 128).transpose(2, 1, 0)
        .astype(np.float16)
    )
    W16 = np.asarray(W, np.float32).astype(np.float16)
    B16 = np.asarray(lora_B, np.float32).astype(np.float16)
    b16 = np.asarray(b, np.float32).astype(np.float16)
    in_maps = []
    for i in range(NCORES):
        sl = slice(i * DC, (i + 1) * DC)
        # wt[c, p, s*DC + n] = W[DC*i + n, 128*(KCHUNK*c+s) + p]
        wt = np.ascontiguousarray(
            W16[sl, :].T.reshape(NCHUNK, KCHUNK, 128, DC).transpose(0, 2, 1, 3)
            .reshape(NCHUNK, 128, KCHUNK * DC)
        )
        bb = np.empty((R + 1, DC), np.float16)
        bb[:R] = B16[sl, :].T
        bb[R] = b16[sl]
        in_maps.append({"xt": xt, "at": at, "wt": wt, "bb": bb})
    return in_maps


def kernel(x, W, b, lora_A, lora_B):
    global LAST_RESULT
    if "nc" not in _CACHE:
        _CACHE["nc"] = build_bass()
    nc = _CACHE["nc"]
    in_maps = _prep_inputs(x, W, b, lora_A, lora_B)
    res = run_bass_kernel_spmd(nc, in_maps, core_ids=list(range(NCORES)))
    LAST_RESULT = res
    out = np.concatenate([res.results[i]["out"] for i in range(NCORES)], axis=1)
    return np.ascontiguousarray(out.reshape(8, 16, DOUT), dtype=np.float32)
